# revision 1
# baseline (speedup 1.0000x reference)
"""Binarized 3x3 conv + batchnorm(train) + sign, on 8 TRN2 NeuronCores.

Math: out = sign((y - mean)/sqrt(var+eps)) where y = conv(x, sign(w)) + sign(b)
and mean/var are per-channel batch stats.  Since sqrt(var+eps) > 0, the output
is exactly sign(y - mean_c): variance never needs to be computed.  The +-1
channel bias cancels in sign(y - mean), so it is dropped entirely.

Strategy (data-parallel over batch, 4 images/core):
 - 1-D Winograd F(2,3) along W (host-side input transform in fp32 during
   prep): 1.5x fewer MACs than direct conv.  d~[j] = B^T d per output-column
   pair (27 pairs), kernel g~ = G g in {+-1, +-1/2, +-3/2} (exact in e4m3),
   vertical taps stay direct (3 kh).  y_even = m0+m1+m2, y_odd = m1-m2-m3.
 - all matmuls fp8-e4m3 DoubleRow (0.5 PE cycles/output-row), 243-wide
   contiguous rhs slices (no wrap waste), 36 matmuls per output tile
   (4 j x 3 comps x 3 kh) into 4 PSUM accumulators.
 - fp32-quality via the 3-component split d~ ~= c1 + c2/64 + c3/64 with the
   /64 folded into the weights (g~, g~/64).  Measured on the reference
   inputs: 148/23.9M sign flips (rel err 5.0e-3).
 - drains combine the 4 j-accumulators into y_sb even|odd halves on
   VectorE/GpSimd, harvesting per-channel sums for the mean for free.
 - one tiny AllReduce (128x2 fp32) across the 8 cores for the global mean.
 - binarize (y + (-mean)) >= 0 on VectorE/GpSimd as fp8 {0,1} bytes, the
   last block per-tile so only one small chain trails the PE stream; host
   de-interleaves even/odd and maps to +-1 fp32.
"""

import sys

if "/opt/trn_rl_repo" not in sys.path:
    sys.path.insert(0, "/opt/trn_rl_repo")

import numpy as np
import ml_dtypes

N_CORES = 8
N_PER_CORE = 4          # images per core
CI = 256                # in channels
CO = 256                # out channels
H = W = 56
OH = OW = 54
NPIX = OH * OW          # 2916
RT = 3                  # row tiles per image (18 rows each)
RROWS = 18
NP_ = 27                # output column pairs
JP = 4                  # Winograd positions
TFREE = RROWS * NP_     # 243 outputs per tile half (even or odd)
N_TOT = N_CORES * N_PER_CORE
MEAN_SCALE = 1.0 / (N_TOT * NPIX)
C_SCALE = 64.0          # residual components stored at 64x, weights at 1/64
NT = N_PER_CORE * 2 * RT  # 48 tiles per core
# head rows 0-37 (tiles 0-1), tail rows 36-55 (tile 2); per-j planes padded
# so the ci-block pair strides stay 16B-aligned
HROWS = 38
TROWS = 20
TAIL_R0 = 36
JPAD = 1040             # head: 38*27=1026 -> 1040
JPADT = 544             # tail: 20*27=540 -> 544

# binarize engine per block (cb*4+n): v=DVE is_ge {0,1}, a=ScalarE Sign
# {-1,0,1}, g=GpSimd is_ge; block 7 runs per-tile on DVE
BIN_ENG = ["v", "a", "g", "v", "a", "g", "a", "v"]

FP8 = ml_dtypes.float8_e4m3


def build(nc, n_cores=N_CORES):
    """Emit the SPMD program into a bacc.Bacc instance."""
    import concourse.mybir as mybir
    from concourse import tile

    f32 = mybir.dt.float32
    fp8 = mybir.dt.float8e4
    DR = mybir.MatmulPerfMode.DoubleRow
    ACT = mybir.ActivationFunctionType
    ADD = mybir.AluOpType.add
    MUL = mybir.AluOpType.mult

    xh_d = [
        nc.dram_tensor(f"xh{c}", [N_PER_CORE, 128, 2, JP * JPAD], fp8, kind="ExternalInput")
        for c in range(3)
    ]
    xt_d = [
        nc.dram_tensor(f"xt{c}", [N_PER_CORE, 128, 2, JP * JPADT], fp8, kind="ExternalInput")
        for c in range(3)
    ]
    w1_d = nc.dram_tensor("w1", [128, 2, JP, 3, 2, 128], fp8, kind="ExternalInput")
    ws_d = nc.dram_tensor("ws", [128, 2, JP, 3, 2, 128], fp8, kind="ExternalInput")
    y_d = nc.dram_tensor("y", [N_PER_CORE, 2, 128, NPIX], mybir.dt.uint8, kind="ExternalOutput")

    with tile.TileContext(nc) as tc:
        with (
            tc.tile_pool(name="wpool", bufs=1) as wpool,
            tc.tile_pool(name="xpool", bufs=2) as xpool,
            tc.tile_pool(name="ypool", bufs=1) as ypool,
            tc.tile_pool(name="spool", bufs=1) as spool,
            tc.tile_pool(name="opool", bufs=2) as opool,
            tc.tile_pool(name="btpool", bufs=6) as btpool,
            tc.tile_pool(name="pspool", bufs=8, space="PSUM") as pspool,
            tc.tile_pool(name="drampool", bufs=2, space="DRAM") as drampool,
        ):
            w1_sb = wpool.tile([128, 2, JP, 3, 2, 128], fp8, tag="w1")
            ws_sb = wpool.tile([128, 2, JP, 3, 2, 128], fp8, tag="ws")
            y_sb = ypool.tile([128, NT * 2 * TFREE], f32)
            sums = spool.tile([128, 4 * NT], f32, tag="sums")
            sums2 = spool.tile([128, 2], f32, tag="sums2")
            neg_mean = spool.tile([128, 2], f32, tag="negmean")

            def emit_mean_cb(cb):
                # cb's per-channel mean: reduce its 48 sum cols + scale.
                # (In the 8-core path the AllReduce for this half follows in
                # phase 2; the timed single-core path uses sums2 directly.)
                nc.vector.tensor_reduce(
                    sums2[:, cb : cb + 1],
                    sums[:, cb * 48 : (cb + 1) * 48].rearrange(
                        "p (a m) -> p a m", a=1
                    ),
                    axis=mybir.AxisListType.X,
                    op=ADD,
                )
                if n_cores == 1:
                    nc.vector.tensor_scalar(
                        neg_mean[:, cb : cb + 1],
                        sums2[:, cb : cb + 1],
                        -MEAN_SCALE, 0.0, MUL, ADD,
                    )

            # ---------------- phase 1: conv + drain (+sums) ------------------
            # HWDGE serializes all transfers; order startup by first use.
            # First 3-tile group runs component-major so the opening matmuls
            # need only w1[cb0] + comp-1 heads (j01 first).
            for n in range(N_PER_CORE):
                xh = [
                    xpool.tile([128, 2, JP * JPAD], fp8, tag=f"xh{c}", name=f"xh{c}")
                    for c in range(3)
                ]
                xt = [
                    xpool.tile([128, 2, JP * JPADT], fp8, tag=f"xt{c}", name=f"xt{c}")
                    for c in range(3)
                ]
                if n == 0:
                    nc.sync.dma_start(w1_sb[:, 0], w1_d[:, 0])
                    nc.scalar.dma_start(xh[0][:, :, 0 : 2 * JPAD], xh_d[0][n][:, :, 0 : 2 * JPAD])
                    nc.sync.dma_start(ws_sb[:, 0], ws_d[:, 0])
                    nc.scalar.dma_start(xh[1][:, :, 0 : 2 * JPAD], xh_d[1][n][:, :, 0 : 2 * JPAD])
                    nc.sync.dma_start(xh[0][:, :, 2 * JPAD :], xh_d[0][n][:, :, 2 * JPAD :])
                    nc.scalar.dma_start(xh[2][:, :, 0 : 2 * JPAD], xh_d[2][n][:, :, 0 : 2 * JPAD])
                    nc.sync.dma_start(xh[1][:, :, 2 * JPAD :], xh_d[1][n][:, :, 2 * JPAD :])
                    nc.scalar.dma_start(xt[0][:], xt_d[0][n])
                    nc.sync.dma_start(xh[2][:, :, 2 * JPAD :], xh_d[2][n][:, :, 2 * JPAD :])
                    nc.scalar.dma_start(xt[1][:], xt_d[1][n])
                    nc.sync.dma_start(xt[2][:], xt_d[2][n])
                    nc.sync.dma_start(w1_sb[:, 1], w1_d[:, 1])
                    nc.scalar.dma_start(ws_sb[:, 1], ws_d[:, 1])
                else:
                    nc.sync.dma_start(xh[0][:], xh_d[0][n])
                    nc.sync.dma_start(xt[0][:], xt_d[0][n])
                    nc.sync.dma_start(xh[1][:], xh_d[1][n])
                    nc.sync.dma_start(xt[1][:], xt_d[1][n])
                    nc.sync.dma_start(xh[2][:], xh_d[2][n])
                    nc.sync.dma_start(xt[2][:], xt_d[2][n])

                def emit_mm(ps_t, cb, rt, j, c, kh):
                    w_sb = w1_sb if c == 0 else ws_sb
                    row = rt * RROWS + kh
                    if rt < 2:
                        src, base, jp = xh[c], 0, JPAD
                    else:
                        src, base, jp = xt[c], TAIL_R0, JPADT
                    off = j * jp + (row - base) * NP_
                    nc.tensor.matmul(
                        ps_t[j][:, 0:TFREE],
                        w_sb[:, cb, j, kh],
                        src[:, :, off : off + TFREE],
                        start=(c == 0 and kh == 0),
                        stop=(c == 2 and kh == 2),
                        perf_mode=DR,
                    )

                def emit_drain(ps_t, cb, rt):
                    t = (cb * N_PER_CORE + n) * RT + rt
                    ev = y_sb[:, 2 * t * TFREE : (2 * t + 1) * TFREE]
                    od = y_sb[:, (2 * t + 1) * TFREE : (2 * t + 2) * TFREE]
                    p0 = ps_t[0][:, 0:TFREE]
                    p1 = ps_t[1][:, 0:TFREE]
                    p2 = ps_t[2][:, 0:TFREE]
                    p3 = ps_t[3][:, 0:TFREE]
                    # HW: only ONE PSUM operand per DVE op, and GpSimd
                    # cannot read PSUM at all.  ScalarE copies j0/j1/j2 out
                    # (harvesting their sums A,B,C), GpSimd adds the even
                    # half in SBUF, DVE adds the odd half (PSUM j3 last,
                    # harvesting O).  Sigma-y per tile = A+B+C+O.
                    s2 = btpool.tile([128, TFREE], f32, tag="s2", name="s2")
                    nc.scalar.activation(
                        ev, p0, ACT.Copy, accum_out=sums[:, 4 * t : 4 * t + 1]
                    )
                    nc.scalar.activation(
                        od, p1, ACT.Copy, accum_out=sums[:, 4 * t + 1 : 4 * t + 2]
                    )
                    nc.scalar.activation(
                        s2[:], p2, ACT.Copy, accum_out=sums[:, 4 * t + 2 : 4 * t + 3]
                    )
                    nc.vector.tensor_add(ev, ev, od)
                    nc.vector.tensor_add(ev, ev, s2[:])
                    nc.vector.tensor_sub(od, od, s2[:])
                    nc.vector.scalar_tensor_tensor(
                        od, p3, -1.0, od, MUL, ADD,
                        accum_out=sums[:, 4 * t + 3 : 4 * t + 4],
                    )

                if n == 0:
                    # cb0 runs component-major so each comp's transfers
                    # stream in just ahead of their first use
                    pss = [
                        [
                            pspool.tile(
                                [128, 512], f32, tag="ps", name=f"ps{i}_{j}"
                            )
                            for j in range(JP)
                        ]
                        for i in range(2)
                    ]
                    for c in range(3):
                        for j in range(JP):
                            for kh in range(3):
                                for i in range(2):
                                    emit_mm(pss[i], 0, i, j, c, kh)
                    for i in range(2):
                        emit_drain(pss[i], 0, i)
                    rest = [(0, 2)] + [(1, rt) for rt in range(RT)]
                else:
                    rest = [(cb, rt) for cb in range(2) for rt in range(RT)]

                for cb, rt in rest:
                    ps = [
                        pspool.tile([128, 512], f32, tag="ps", name=f"psj{j}")
                        for j in range(JP)
                    ]
                    for j in range(JP):
                        for c in range(3):
                            for kh in range(3):
                                emit_mm(ps, cb, rt, j, c, kh)
                    emit_drain(ps, cb, rt)
                    if n == N_PER_CORE - 1 and cb == 0 and rt == RT - 1:
                        # all cb0 sums are in: compute its mean now (DVE is
                        # idle-waiting on cb1 stops here) so GpSimd's cb0
                        # binarize blocks overlap the remaining PE work
                        emit_mean_cb(0)

            # ---------------- phase 2: global mean via AllReduce ------------
            # per-co-block: cb0's sums are complete ~10us before cb1's, so
            # cb0's mean (reduced on the queue-clear GpSimd) unblocks its
            # binarize blocks while the PE still computes cb1.
            emit_mean_cb(1)
            if n_cores > 1:
                for cb in range(2):
                    cc_in = drampool.tile([128, 1], f32, name=f"ccin{cb}")
                    cc_out = drampool.tile([128, 1], f32, name=f"ccout{cb}")
                    nc.sync.dma_start(cc_in[:], sums2[:, cb : cb + 1])
                    nc.gpsimd.collective_compute(
                        "AllReduce",
                        ADD,
                        replica_groups=[list(range(n_cores))],
                        ins=[cc_in.opt()],
                        outs=[cc_out.opt()],
                    )
                    sums_g = spool.tile([128, 1], f32, tag=f"sumsg{cb}")
                    nc.sync.dma_start(sums_g[:], cc_out[:])
                    nc.vector.tensor_scalar(
                        neg_mean[:, cb : cb + 1], sums_g[:],
                        -MEAN_SCALE, 0.0, MUL, ADD,
                    )

            # ---------------- phase 3: binarize + store ---------------------
            # blocks spread over DVE / ScalarE(Sign) / GpSimd per BIN_ENG;
            # the last block runs per-tile on DVE.
            IS_GE = mybir.AluOpType.is_ge
            for b in range(2 * N_PER_CORE):
                cb, n = divmod(b, N_PER_CORE)
                t0 = b * RT
                nm = neg_mean[:, cb : cb + 1]
                if b == 2 * N_PER_CORE - 1:
                    for i in range(RT):
                        t = t0 + i
                        bt = btpool.tile([128, 2 * TFREE], fp8, tag="bint")
                        nc.vector.tensor_scalar(
                            bt[:],
                            y_sb[:, 2 * t * TFREE : (2 * t + 2) * TFREE],
                            nm, 0.0, ADD, IS_GE,
                        )
                        nc.sync.dma_start(
                            y_d[n, cb][:, i * 2 * TFREE : (i + 1) * 2 * TFREE],
                            bt[:].bitcast(mybir.dt.uint8),
                        )
                else:
                    e = nc.gpsimd if cb == 0 else nc.vector
                    bin_t = opool.tile([128, RT * 2 * TFREE], fp8, tag="bin")
                    e.tensor_scalar(
                        bin_t[:],
                        y_sb[:, 2 * t0 * TFREE : 2 * (t0 + RT) * TFREE],
                        nm, 0.0, ADD, IS_GE,
                    )
                    nc.sync.dma_start(y_d[n, cb], bin_t[:].bitcast(mybir.dt.uint8))

    nc.compile()
    return nc


def prep_inputs(x, weight, bias):
    """Host-side shard + Winograd transform + fp8 split."""
    assert x.shape == (N_TOT, CI, H, W) and x.dtype == np.float32

    xs = np.ascontiguousarray(
        x.reshape(N_CORES, N_PER_CORE, 2, 128, H, W).transpose(0, 1, 3, 2, 4, 5)
    )  # [core, n, ci_f, ci_b, 56, 56]
    a = xs[..., 0:54:2]
    b = xs[..., 1:55:2]
    c = xs[..., 2:56:2]
    d = xs[..., 3:56:2]
    dt = np.stack([a - c, b + c, c - b, b - d], axis=4)  # [.., ci_b, j, 56h, 27]

    c1 = dt.astype(FP8)
    r1 = dt - c1.astype(np.float32)
    c2 = (r1 * np.float32(C_SCALE)).astype(FP8)
    r2 = r1 - c2.astype(np.float32) * np.float32(1.0 / C_SCALE)
    c3 = (r2 * np.float32(C_SCALE)).astype(FP8)

    def halves(cq):
        # [core, n, 128, 2, 4, 56, 27] -> head rows 0-37, tail rows 36-55
        hd = cq[..., 0:HROWS, :].reshape(N_CORES, N_PER_CORE, 128, 2, JP, HROWS * NP_)
        tl = cq[..., TAIL_R0:, :].reshape(N_CORES, N_PER_CORE, 128, 2, JP, TROWS * NP_)
        hd = np.pad(hd, ((0, 0),) * 5 + ((0, JPAD - HROWS * NP_),))
        tl = np.pad(tl, ((0, 0),) * 5 + ((0, JPADT - TROWS * NP_),))
        return (
            hd.reshape(N_CORES, N_PER_CORE, 128, 2, JP * JPAD),
            tl.reshape(N_CORES, N_PER_CORE, 128, 2, JP * JPADT),
        )

    hs, ts = zip(*(halves(q) for q in (c1, c2, c3)))

    wb = np.where(weight >= 0, np.float32(1.0), np.float32(-1.0))
    g0 = wb[:, :, :, 0]
    g1 = wb[:, :, :, 1]
    g2 = wb[:, :, :, 2]
    gt = np.stack(
        [g0, (g0 + g1 + g2) / 2, (g0 - g1 + g2) / 2, g2], axis=3
    ).astype(np.float32)  # [co, ci, kh, j]
    # [co_b, co_f, ci_b, ci_f, kh, j] -> [ci_f, co_b, j, kh, ci_b, co_f]
    g6 = gt.reshape(2, 128, 2, 128, 3, JP)
    wt = np.ascontiguousarray(g6.transpose(3, 0, 5, 4, 2, 1))
    w1 = wt.astype(FP8)
    ws = (wt * np.float32(1.0 / C_SCALE)).astype(FP8)
    assert np.all(w1.astype(np.float32) == wt)
    assert np.all(ws.astype(np.float32) * C_SCALE == wt)

    out = []
    for core in range(N_CORES):
        m = {"w1": w1, "ws": ws}
        for ci in range(3):
            m[f"xh{ci}"] = hs[ci][core]
            m[f"xt{ci}"] = ts[ci][core]
        out.append(m)
    return out


def gather(results):
    """[{y: [4,2,128,2916] fp8 {0,1}}] * 8 -> (32, 256, 54, 54) fp32 +-1.

    Per row-tile the 486 bytes are [even 9x27 | odd 9x27]; de-interleave."""
    ys = np.stack([np.asarray(r["y"]).view(FP8) for r in results]).astype(np.float32)
    ys = ys.reshape(N_CORES, N_PER_CORE, 2, 128, RT, 2, RROWS, NP_)
    out = np.empty((N_CORES, N_PER_CORE, 2, 128, RT, RROWS, OW), np.float32)
    out[..., 0::2] = ys[:, :, :, :, :, 0]
    out[..., 1::2] = ys[:, :, :, :, :, 1]
    return out.reshape(N_TOT, CO, OH, OW) * np.float32(2.0) - np.float32(1.0)


_STATE = {}


def _get_nc():
    if "nc" not in _STATE:
        import concourse.bacc as bacc

        nc = bacc.Bacc(
            "TRN2", target_bir_lowering=False, debug=False, num_devices=N_CORES
        )
        _STATE["nc"] = build(nc)
    return _STATE["nc"]


def kernel(x, weight, bias, _trace=False):
    from concourse.bass_utils import run_bass_kernel_spmd

    nc = _get_nc()
    in_maps = prep_inputs(
        np.asarray(x, np.float32),
        np.asarray(weight, np.float32),
        np.asarray(bias, np.float32),
    )
    res = run_bass_kernel_spmd(
        nc, in_maps, core_ids=list(range(N_CORES)), trace=_trace
    )
    _STATE["last_result"] = res
    return gather(res.results)



# revision 24
# speedup vs baseline: 1.1642x; 1.1642x over previous
"""Binarized 3x3 conv + batchnorm(train) + sign, on 8 TRN2 NeuronCores.

Math: out = sign((y - mean)/sqrt(var+eps)) where y = conv(x, sign(w)) + sign(b)
and mean/var are per-channel batch stats.  Since sqrt(var+eps) > 0, the output
is exactly sign(y - mean_c): variance never needs to be computed.  The +-1
channel bias cancels in sign(y - mean), so it is dropped entirely.

Strategy (data-parallel over batch, 4 images/core):
 - 1-D Winograd F(4,3) along W (host-side input transform in fp32 during
   prep): 6 points {0, +-1, +-2, inf} per 4 output columns, 14 groups cover
   56 output cols (the last 2 are cropped on the host).  Weight transform
   G = Vandermonde([1,p,p^2]) keeps every binarized-weight combo exact in
   e4m3 ({+-1,+-3,+-5,+-7} etc).  The +-2 data rows are pre-scaled by 8
   (power of two) to clear the fp8 subnormal floor; the drain copies apply
   the compensating 1/8.  Vertical taps stay direct (3 kh).
 - all matmuls fp8-e4m3 DoubleRow (contraction 256 = 2x128 ci for free),
   252-wide rhs slices (18 rows x 14 groups), 54 matmuls per tile
   (6 j x 3 kh x 3 comps) into 6 sub-bank PSUM accumulators.
 - fp32-quality via the 3-component split d~ ~= c1 + c2/64 + c3/64 with the
   /64 folded into the ws weight copy.  Measured on the reference inputs:
   flip rate 1.8e-5 (rel err ~8.5e-3).
 - drains: ScalarE copies m1, m3/8, m4/8 out of PSUM (harvesting channel
   sums), DVE/GpSimd combine them into y0..y3 via the inverse transform
   A^T = [[1,1,1,s,s,0],[0,1,-1,2s,-2s,0],[0,1,1,4s,4s,0],[0,1,-1,8s,8s,1]]
   (s = 1/8), harvesting enough accumulator sums to reconstruct the channel
   mean (including a correction for the 2 cropped columns).
 - one tiny AllReduce (128x2 fp32) across the 8 cores for the global mean.
 - binarize (y + (-mean)) >= 0 spread over DVE / ScalarE(Sign) / GpSimd,
   emitted right after each co-half's mean so cb0's blocks overlap the
   remaining cb1 matmuls; host decodes {0,1} or {-1,0,1} per block.
"""

import sys

if "/opt/trn_rl_repo" not in sys.path:
    sys.path.insert(0, "/opt/trn_rl_repo")

import numpy as np
import ml_dtypes

N_CORES = 8
N_PER_CORE = 4          # images per core
CI = 256                # in channels
CO = 256                # out channels
H = W = 56
OH = OW = 54
GP = 14                 # F(4,3) groups along W (56 out cols, last 2 cropped)
JP = 6                  # Winograd points {0, 1, -1, 2, -2, inf}
RT = 3                  # row tiles per image (18 rows each)
RROWS = 18
TFREE = RROWS * GP      # 252 outputs per j-plane per tile
JSTR = 256              # psum accumulator stride in f32 elems (1KB half-bank)
YW = 4 * TFREE          # 1008 y px per tile (56 cols incl the 2 cropped)
NT = N_PER_CORE * 2 * RT  # 24 tiles per core
N_TOT = N_CORES * N_PER_CORE
MEAN_SCALE = 1.0 / (N_TOT * OH * OW)
C_SCALE = 64.0          # residual components stored at 64x, weights at 1/64
S3 = 8.0                # host scales B^T rows 3,4 (points +-2) by 8
# head rows 0-37 (tiles 0-1), tail rows 36-55 (tile 2)
HROWS = 38
TROWS = 20
TAIL_R0 = 36
HPAD = 536              # head: 38*14=532 -> 536
TPAD = 288              # tail: 20*14=280 -> 288
NQ = 7                  # harvested sum quantities: H,V,C,E,F2,G2,ZG
QH, QV, QC, QE, QF, QG, QZ = range(NQ)
# physical j-plane order (host permutes): banks (0,1),(2,3),(4,5) hold
# logical points (m1,m3),(m4,m2),(m0,m5); matmul groups run in physical
# order so the bank a drain copy needs first is retired first
JPERM = [1, 3, 4, 2, 0, 5]   # physical slot -> logical point
JSLOT = {1: 0, 3: 1, 4: 2, 2: 3, 0: 4, 5: 5}  # logical point -> slot

# binarize engine per (cb, n, rt) chunk: v=DVE is_ge {0,1}, a=ScalarE Sign
# {-1,0,1}, g=GpSimd is_ge.  cb0 chunks run under the deferred-tile window
# where ScalarE has the most slack; cb1 chunks run post-PE where DVE's 2x
# mode makes it the fastest engine.
BIN_PAT = {0: ["aga", "vga", "aag", "aga"], 1: ["vav", "vga", "vav", "vga"]}

FP8 = ml_dtypes.float8_e4m3

# F(4,3) transforms: points {0, 1, -1, 2, -2, inf}; B^T rows 3,4 scaled by S3.
_BT = np.array([
    [1.0,    0.0,   -1.25,   0.0,    0.25,  0.0],
    [0.0,  2/3.0,   2/3.0, -1/6.0, -1/6.0,  0.0],
    [0.0, -2/3.0,   2/3.0,  1/6.0, -1/6.0,  0.0],
    [0.0, -2/3.0,  -1/3.0,  2/3.0,  1/3.0,  0.0],   # x8 applied
    [0.0,  2/3.0,  -1/3.0, -2/3.0,  1/3.0,  0.0],   # x8 applied
    [0.0,    4.0,     0.0,   -5.0,   0.0,   1.0],
], dtype=np.float64)
_G = np.array([
    [1.0,  0.0, 0.0],
    [1.0,  1.0, 1.0],
    [1.0, -1.0, 1.0],
    [1.0,  2.0, 4.0],
    [1.0, -2.0, 4.0],
    [0.0,  0.0, 1.0],
], dtype=np.float64)


def build(nc, n_cores=N_CORES):
    """Emit the SPMD program into a bacc.Bacc instance."""
    import concourse.mybir as mybir
    from concourse import tile

    f32 = mybir.dt.float32
    fp8 = mybir.dt.float8e4
    DR = mybir.MatmulPerfMode.DoubleRow
    ACT = mybir.ActivationFunctionType
    ADD = mybir.AluOpType.add
    MUL = mybir.AluOpType.mult
    IS_GE = mybir.AluOpType.is_ge

    xh_d = [
        nc.dram_tensor(f"xh{c}", [N_PER_CORE, 128, 2, JP, HPAD], fp8, kind="ExternalInput")
        for c in range(3)
    ]
    xt_d = [
        nc.dram_tensor(f"xt{c}", [N_PER_CORE, 128, 2, JP, TPAD], fp8, kind="ExternalInput")
        for c in range(3)
    ]
    w1_d = nc.dram_tensor("w1", [128, 2, JP, 3, 2, 128], fp8, kind="ExternalInput")
    ws_d = nc.dram_tensor("ws", [128, 2, JP, 3, 2, 128], fp8, kind="ExternalInput")
    y_d = nc.dram_tensor("y", [N_PER_CORE, 2, 128, RT * YW], mybir.dt.uint8, kind="ExternalOutput")

    with tile.TileContext(nc) as tc:
        with (
            tc.tile_pool(name="wpool", bufs=1) as wpool,
            tc.tile_pool(name="xpool", bufs=2) as xpool,
            tc.tile_pool(name="ypool", bufs=1) as ypool,
            tc.tile_pool(name="spool", bufs=1) as spool,
            tc.tile_pool(name="opool", bufs=1) as opool,
            tc.tile_pool(name="btpool", bufs=3) as btpool,
            tc.tile_pool(name="ctpool", bufs=2) as ctpool,
            tc.tile_pool(name="pspool", bufs=2, space="PSUM") as pspool,
            tc.tile_pool(name="drampool", bufs=2, space="DRAM") as drampool,
        ):
            w1_sb = wpool.tile([128, 2, JP, 3, 2, 128], fp8, tag="w1")
            ws_sb = wpool.tile([128, 2, JP, 3, 2, 128], fp8, tag="ws")
            y_sb = ypool.tile([128, NT * YW], f32)
            sums = spool.tile([128, NQ * NT], f32, tag="sums")
            obuf_t = opool.tile([128, N_PER_CORE * RT * YW], fp8, tag="bin")
            obuf = obuf_t[:]
            sums2 = spool.tile([128, 2, NQ], f32, tag="sums2")
            acc = spool.tile([128, 2], f32, tag="acc")
            neg_mean = spool.tile([128, 2], f32, tag="negmean")

            def emit_mean_cb(cb):
                # reduce this cb's 12 tile-columns per quantity, then combine:
                # sum(y) = F2 + G2 + H + V + 15C - 5E - ZG  (ZG = cropped cols)
                for q in range(NQ):
                    i0 = q * NT + cb * (NT // 2)
                    nc.vector.tensor_reduce(
                        sums2[:, cb, q : q + 1],
                        sums[:, i0 : i0 + NT // 2].rearrange(
                            "p (a m) -> p a m", a=1
                        ),
                        axis=mybir.AxisListType.X,
                        op=ADD,
                    )
                s2 = lambda q: sums2[:, cb, q : q + 1]
                a = acc[:, cb : cb + 1]
                nc.vector.tensor_add(a, s2(QF), s2(QG))
                nc.vector.tensor_add(a, a, s2(QH))
                nc.vector.tensor_add(a, a, s2(QV))
                nc.vector.scalar_tensor_tensor(a, s2(QC), 15.0, a, MUL, ADD)
                nc.vector.scalar_tensor_tensor(a, s2(QE), -5.0, a, MUL, ADD)
                nc.vector.scalar_tensor_tensor(a, s2(QZ), -1.0, a, MUL, ADD)
                if n_cores == 1:
                    nc.vector.tensor_scalar(
                        neg_mean[:, cb : cb + 1], a, -MEAN_SCALE, 0.0, MUL, ADD,
                    )
                else:
                    cc_in = drampool.tile([128, 1], f32, name=f"ccin{cb}")
                    cc_out = drampool.tile([128, 1], f32, name=f"ccout{cb}")
                    nc.sync.dma_start(cc_in[:], a)
                    nc.gpsimd.collective_compute(
                        "AllReduce",
                        ADD,
                        replica_groups=[list(range(n_cores))],
                        ins=[cc_in.opt()],
                        outs=[cc_out.opt()],
                    )
                    sums_g = spool.tile([128, 1], f32, tag=f"sumsg{cb}")
                    nc.sync.dma_start(sums_g[:], cc_out[:])
                    nc.vector.tensor_scalar(
                        neg_mean[:, cb : cb + 1], sums_g[:],
                        -MEAN_SCALE, 0.0, MUL, ADD,
                    )

            def emit_bin_tile(cb, bn, i, per_tile_dma=False):
                eng = BIN_PAT[cb][bn][i]
                nm = neg_mean[:, cb : cb + 1]
                bt = obuf[:, (bn * RT + i) * YW : (bn * RT + i + 1) * YW]
                ysl = y_sb[:, ((cb * N_PER_CORE + bn) * RT + i) * YW
                           : ((cb * N_PER_CORE + bn) * RT + i + 1) * YW]
                if eng == "a":
                    nc.scalar.activation(bt, ysl, ACT.Sign, bias=nm)
                elif eng == "g":
                    nc.gpsimd.tensor_scalar(bt, ysl, nm, 0.0, ADD, IS_GE)
                else:
                    nc.vector.tensor_scalar(bt, ysl, nm, 0.0, ADD, IS_GE)
                oq = (nc.sync, nc.scalar, nc.gpsimd)[bn % 3]
                if per_tile_dma:
                    oq.dma_start(
                        y_d[bn, cb][:, i * YW : (i + 1) * YW],
                        bt.bitcast(mybir.dt.uint8),
                    )
                elif i == RT - 1:
                    oq.dma_start(
                        y_d[bn, cb],
                        obuf[:, bn * RT * YW : (bn + 1) * RT * YW].bitcast(
                            mybir.dt.uint8
                        ),
                    )

            pending_bin = []

            # ---------------- phase 1: conv + drain (+sums) ------------------
            x_tiles = {}

            def load_image(n):
                xh = [
                    xpool.tile([128, 2, JP, HPAD], fp8, tag=f"xh{c}", name=f"xh{c}")
                    for c in range(3)
                ]
                xt = [
                    xpool.tile([128, 2, JP, TPAD], fp8, tag=f"xt{c}", name=f"xt{c}")
                    for c in range(3)
                ]
                if n == 0:
                    # cold start runs the TAIL tile first: it only needs
                    # w1[cb0] + the (smaller) xt planes, so the PE starts
                    # ~3us sooner while the big head planes stream in
                    nc.sync.dma_start(w1_sb[:, 0, 0], w1_d[:, 0, 0])
                    nc.scalar.dma_start(xt[0][:, :, 0:1], xt_d[0][n][:, :, 0:1])
                    nc.gpsimd.dma_start(xt[1][:, :, 0:1], xt_d[1][n][:, :, 0:1])
                    nc.sync.dma_start(ws_sb[:, 0, 0], ws_d[:, 0, 0])
                    nc.scalar.dma_start(xt[2][:, :, 0:1], xt_d[2][n][:, :, 0:1])
                    nc.gpsimd.dma_start(w1_sb[:, 0, 1:], w1_d[:, 0, 1:])
                    nc.sync.dma_start(xt[0][:, :, 1:], xt_d[0][n][:, :, 1:])
                    nc.scalar.dma_start(ws_sb[:, 0, 1:], ws_d[:, 0, 1:])
                    nc.gpsimd.dma_start(xt[1][:, :, 1:], xt_d[1][n][:, :, 1:])
                    nc.sync.dma_start(xt[2][:, :, 1:], xt_d[2][n][:, :, 1:])
                    nc.gpsimd.dma_start(xh[0][:, :, 0:3], xh_d[0][n][:, :, 0:3])
                    nc.sync.dma_start(xh[0][:, :, 3:], xh_d[0][n][:, :, 3:])
                    nc.scalar.dma_start(xh[1][:, :, 0:3], xh_d[1][n][:, :, 0:3])
                    nc.gpsimd.dma_start(xh[1][:, :, 3:], xh_d[1][n][:, :, 3:])
                    nc.sync.dma_start(xh[2][:, :, 0:3], xh_d[2][n][:, :, 0:3])
                    nc.scalar.dma_start(xh[2][:, :, 3:], xh_d[2][n][:, :, 3:])
                    nc.sync.dma_start(w1_sb[:, 1], w1_d[:, 1])
                    nc.scalar.dma_start(ws_sb[:, 1], ws_d[:, 1])
                else:
                    nc.sync.dma_start(xh[0][:], xh_d[0][n])
                    nc.scalar.dma_start(xt[0][:], xt_d[0][n])
                    nc.sync.dma_start(xh[1][:], xh_d[1][n])
                    nc.scalar.dma_start(xt[1][:], xt_d[1][n])
                    nc.sync.dma_start(xh[2][:], xh_d[2][n])
                    nc.scalar.dma_start(xt[2][:], xt_d[2][n])
                x_tiles[n] = (xh, xt)

            def emit_mm(xh, xt, ps, cb, rt, p, c, kh):
                # p is the PHYSICAL slot; host already permuted planes
                w_sb = w1_sb if c == 0 else ws_sb
                row = rt * RROWS + kh
                if rt < 2:
                    src, base = xh[c], 0
                else:
                    src, base = xt[c], TAIL_R0
                off = (row - base) * GP
                nc.tensor.matmul(
                    ps[p // 2][:, (p % 2) * JSTR : (p % 2) * JSTR + TFREE],
                    w_sb[:, cb, p, kh],
                    src[:, :, p, off : off + TFREE],
                    start=(c == 0 and kh == 0),
                    stop=(c == 2 and kh == 2),
                    perf_mode=DR,
                )

            def emit_drain(n, ps, cb, rt):
                ti = (cb * N_PER_CORE + n) * RT + rt
                q = lambda qi: sums[:, qi * NT + ti : qi * NT + ti + 1]
                mj = lambda j: ps[JSLOT[j] // 2][
                    :, (JSLOT[j] % 2) * JSTR : (JSLOT[j] % 2) * JSTR + TFREE
                ]
                ysl = lambda t: y_sb[:, (ti * 4 + t) * TFREE : (ti * 4 + t + 1) * TFREE]
                t1 = ctpool.tile([128, TFREE], f32, tag="t1", name="t1")
                t3 = ctpool.tile([128, TFREE], f32, tag="t3", name="t3")
                t4 = ctpool.tile([128, TFREE], f32, tag="t4", name="t4")
                u1 = btpool.tile([128, TFREE], f32, tag="u1", name="u1")
                v1 = btpool.tile([128, TFREE], f32, tag="v1", name="v1")
                u2 = btpool.tile([128, TFREE], f32, tag="u2", name="u2")
                v2 = btpool.tile([128, TFREE], f32, tag="v2", name="v2")
                w0 = btpool.tile([128, TFREE], f32, tag="w0", name="w0")
                w5 = btpool.tile([128, TFREE], f32, tag="w5", name="w5")
                zg = ctpool.tile([128, RROWS], f32, tag="zg", name="zg")
                # PSUM reads stay within 1 cross-engine hop of the PE so
                # the accumulator set frees ~a single op-chain after the
                # tile's last matmul: Act copies banks 0-1, DVE combines
                # bank 1-2 into local w0 = m0+u1, w5 = m5+v1.
                nc.scalar.activation(t1[:], mj(1), ACT.Copy)
                nc.scalar.activation(t3[:], mj(3), ACT.Copy, scale=1.0 / S3, accum_out=q(QC))
                nc.scalar.activation(t4[:], mj(4), ACT.Copy, scale=1.0 / S3, accum_out=q(QE))
                nc.vector.scalar_tensor_tensor(u1[:], mj(2), 1.0, t1[:], MUL, ADD, accum_out=q(QH))
                nc.vector.scalar_tensor_tensor(v1[:], mj(2), -1.0, t1[:], MUL, ADD, accum_out=q(QV))
                nc.vector.scalar_tensor_tensor(w0[:], mj(0), 1.0, u1[:], MUL, ADD, accum_out=q(QF))
                nc.vector.scalar_tensor_tensor(w5[:], mj(5), 1.0, v1[:], MUL, ADD, accum_out=q(QG))
                # GpSimd supports only the plain TensorTensor forms on HW;
                # the scaled combines run as DVE STT ops
                nc.gpsimd.tensor_add(u2[:], t3[:], t4[:])
                nc.gpsimd.tensor_sub(v2[:], t3[:], t4[:])
                nc.gpsimd.tensor_add(ysl(0), u2[:], w0[:])
                nc.vector.scalar_tensor_tensor(ysl(1), v2[:], 2.0, v1[:], MUL, ADD)
                nc.vector.scalar_tensor_tensor(ysl(2), u2[:], 4.0, u1[:], MUL, ADD)
                nc.vector.scalar_tensor_tensor(ysl(3), v2[:], 8.0, w5[:], MUL, ADD)
                # cropped-column correction: sum of y2,y3 at group 13
                y2g = ysl(2).rearrange("p (r g) -> p r g", g=GP)[:, :, GP - 1]
                y3g = ysl(3).rearrange("p (r g) -> p r g", g=GP)[:, :, GP - 1]
                nc.vector.scalar_tensor_tensor(zg[:], y2g, 1.0, y3g, MUL, ADD, accum_out=q(QZ))

            # tile schedule: image-2's last two cb1 tiles are deferred past
            # image-3's cb0 so the post-cb0-mean window spans 5 tiles of PE
            # work for the cb0 binarize chunks to hide under
            SCHED = []
            for n in range(N_PER_CORE):
                for cb in range(2):
                    for rt in ([2, 0, 1] if (n == 0 and cb == 0) else range(RT)):
                        SCHED.append((n, cb, rt))
            SCHED.remove((2, 1, 0))
            SCHED.remove((2, 1, 1))
            SCHED.remove((2, 1, 2))
            ins = SCHED.index((3, 0, 2)) + 1
            SCHED[ins:ins] = [(2, 1, 0), (2, 1, 1), (2, 1, 2)]

            pending_bin = []
            for (n, cb, rt) in SCHED:
                if n not in x_tiles:
                    load_image(n)
                xh, xt = x_tiles[n]
                ps = [
                    pspool.tile([128, 2 * JSTR], f32, tag=f"ps{k}", name=f"ps{k}")
                    for k in range(3)
                ]
                # p-major ALWAYS: accumulation groups sharing a PSUM bank
                # must not interleave (a group's start clears the whole
                # bank's has_written bits on HW)
                for p in range(JP):
                    for c in range(3):
                        for kh in range(3):
                            emit_mm(xh, xt, ps, cb, rt, p, c, kh)
                emit_drain(n, ps, cb, rt)
                if (n, cb, rt) == (N_PER_CORE - 1, 0, RT - 1):
                    # all cb0 sums in: compute its mean now; its binarize
                    # chunks interleave with the remaining drains so every
                    # engine queue keeps the PE's PSUM sets moving
                    emit_mean_cb(0)
                    pending_bin = [
                        (0, bn, i)
                        for bn in range(N_PER_CORE)
                        for i in range(RT)
                    ]
                    for _ in range(2):
                        emit_bin_tile(*pending_bin.pop(0))
                elif pending_bin:
                    for _ in range(min(2, len(pending_bin))):
                        emit_bin_tile(*pending_bin.pop(0))

            # ---------------- phase 2: cb1 mean + binarize -------------------
            for args in pending_bin:
                emit_bin_tile(*args)
            emit_mean_cb(1)
            for bn in range(N_PER_CORE):
                for i in range(RT):
                    emit_bin_tile(1, bn, i, per_tile_dma=(bn == N_PER_CORE - 1))

    nc.compile()
    return nc


def prep_inputs(x, weight, bias):
    """Host-side shard + F(4,3) Winograd transform + fp8 3-comp split."""
    assert x.shape == (N_TOT, CI, H, W) and x.dtype == np.float32

    xs = np.ascontiguousarray(
        x.reshape(N_CORES, N_PER_CORE, 2, 128, H, W).transpose(0, 1, 3, 2, 4, 5)
    )  # [core, n, ci_f, ci_b, 56, 56]
    xp = np.pad(xs, ((0, 0),) * 5 + ((0, 2),))  # W 56 -> 58
    idx = np.arange(GP)[:, None] * 4 + np.arange(JP)[None, :]  # [14, 6]
    xg = xp[..., idx]  # [core, n, 128, 2, 56, 14, 6]
    D = np.einsum("jc,...gc->...gj", _BT.astype(np.float32), xg, optimize=True)
    D = np.ascontiguousarray(D.transpose(0, 1, 2, 3, 6, 4, 5)[:, :, :, :, JPERM])
    # [core, n, 128, 2, j(permuted), 56, 14]

    c1 = D.astype(FP8)
    r1 = D - c1.astype(np.float32)
    c2 = (r1 * np.float32(C_SCALE)).astype(FP8)
    r2 = r1 - c2.astype(np.float32) * np.float32(1.0 / C_SCALE)
    c3 = (r2 * np.float32(C_SCALE)).astype(FP8)

    def halves(cq):
        # [core, n, 128, 2, j, 56, 14] -> head rows 0-37, tail rows 36-55
        hd = cq[..., 0:HROWS, :].reshape(N_CORES, N_PER_CORE, 128, 2, JP, HROWS * GP)
        tl = cq[..., TAIL_R0:, :].reshape(N_CORES, N_PER_CORE, 128, 2, JP, TROWS * GP)
        hd = np.pad(hd, ((0, 0),) * 5 + ((0, HPAD - HROWS * GP),))
        tl = np.pad(tl, ((0, 0),) * 5 + ((0, TPAD - TROWS * GP),))
        return hd, tl

    hs, ts = zip(*(halves(q) for q in (c1, c2, c3)))

    wb = np.where(weight >= 0, np.float32(1.0), np.float32(-1.0))
    gt = np.einsum("jk,oihk->oihj", _G.astype(np.float32), wb, optimize=True)
    # [co, ci, kh, j] -> [ci_f, cb, j, kh, ci_b, co_f]
    g6 = gt.reshape(2, 128, 2, 128, 3, JP)
    wt = np.ascontiguousarray(g6.transpose(3, 0, 5, 4, 2, 1)[:, :, JPERM])
    w1 = wt.astype(FP8)
    ws = (wt * np.float32(1.0 / C_SCALE)).astype(FP8)
    assert np.all(w1.astype(np.float32) == wt)
    assert np.all(ws.astype(np.float32) * C_SCALE == wt)

    out = []
    for core in range(N_CORES):
        m = {"w1": w1, "ws": ws}
        for ci in range(3):
            m[f"xh{ci}"] = hs[ci][core]
            m[f"xt{ci}"] = ts[ci][core]
        out.append(m)
    return out


def gather(results):
    """[{y: [4,2,128,3024] u8}] * 8 -> (32, 256, 54, 54) fp32 +-1.

    Per tile the 1008 bytes are 4 y-slices of [18 rows x 14 groups] for
    output column 4g+t; group 13's t=2,3 are cropped.  BIN_PAT[cb][n][rt]=='a'
    chunks were binarized on ScalarE as Sign {-1,0,+1}; the rest are is_ge
    {0,1}."""
    ys = np.stack([np.asarray(r["y"]).view(FP8) for r in results]).astype(np.float32)
    ys = ys.reshape(N_CORES, N_PER_CORE, 2, 128, RT, 4, RROWS, GP)
    out = np.empty_like(ys)
    for cb in range(2):
        for n in range(N_PER_CORE):
            for i in range(RT):
                blk = ys[:, n, cb, :, i]
                out[:, n, cb, :, i] = (
                    np.where(blk < 0, -1.0, 1.0)
                    if BIN_PAT[cb][n][i] == "a"
                    else blk * 2.0 - 1.0
                )
    # [core, n, cb, co_f, rt, t, row, g] -> [core, n, cb, co_f, rt, row, g, t]
    out = out.transpose(0, 1, 2, 3, 4, 6, 7, 5).reshape(
        N_CORES, N_PER_CORE, CO, OH, 4 * GP
    )[..., :OW]
    return np.ascontiguousarray(out.reshape(N_TOT, CO, OH, OW))


_STATE = {}


def _get_nc():
    if "nc" not in _STATE:
        import concourse.bacc as bacc

        nc = bacc.Bacc(
            "TRN2", target_bir_lowering=False, debug=False, num_devices=N_CORES
        )
        _STATE["nc"] = build(nc)
    return _STATE["nc"]


def kernel(x, weight, bias, _trace=False):
    from concourse.bass_utils import run_bass_kernel_spmd

    nc = _get_nc()
    in_maps = prep_inputs(
        np.asarray(x, np.float32),
        np.asarray(weight, np.float32),
        np.asarray(bias, np.float32),
    )
    res = run_bass_kernel_spmd(
        nc, in_maps, core_ids=list(range(N_CORES)), trace=_trace
    )
    _STATE["last_result"] = res
    return gather(res.results)


# revision 28
# speedup vs baseline: 1.1750x; 1.0093x over previous
"""Binarized 3x3 conv + batchnorm(train) + sign, on 8 TRN2 NeuronCores.

Math: out = sign((y - mean)/sqrt(var+eps)) where y = conv(x, sign(w)) + sign(b)
and mean/var are per-channel batch stats.  Since sqrt(var+eps) > 0, the output
is exactly sign(y - mean_c): variance never needs to be computed.  The +-1
channel bias cancels in sign(y - mean), so it is dropped entirely.

Strategy (data-parallel over batch, 4 images/core):
 - 1-D Winograd F(4,3) along W (host-side input transform in fp32 during
   prep): 6 points {0, +-1, +-2, inf} per 4 output columns, 14 groups cover
   56 output cols (the last 2 are cropped on the host).  Weight transform
   G = Vandermonde([1,p,p^2]) keeps every binarized-weight combo exact in
   e4m3 ({+-1,+-3,+-5,+-7} etc).  The +-2 data rows are pre-scaled by 8
   (power of two) to clear the fp8 subnormal floor; the drain copies apply
   the compensating 1/8.  Vertical taps stay direct (3 kh).
 - all matmuls fp8-e4m3 DoubleRow (contraction 256 = 2x128 ci for free),
   252-wide rhs slices (18 rows x 14 groups), 54 matmuls per tile
   (6 j x 3 kh x 3 comps) into 6 sub-bank PSUM accumulators.
 - fp32-quality via the 3-component split d~ ~= c1 + c2/64 + c3/64 with the
   /64 folded into the ws weight copy.  Measured on the reference inputs:
   flip rate 1.8e-5 (rel err ~8.5e-3).
 - drains: ScalarE copies m1, m3/8, m4/8 out of PSUM (harvesting channel
   sums), DVE/GpSimd combine them into y0..y3 via the inverse transform
   A^T = [[1,1,1,s,s,0],[0,1,-1,2s,-2s,0],[0,1,1,4s,4s,0],[0,1,-1,8s,8s,1]]
   (s = 1/8), harvesting enough accumulator sums to reconstruct the channel
   mean (including a correction for the 2 cropped columns).
 - one tiny AllReduce (128x2 fp32) across the 8 cores for the global mean.
 - binarize (y + (-mean)) >= 0 spread over DVE / ScalarE(Sign) / GpSimd,
   emitted right after each co-half's mean so cb0's blocks overlap the
   remaining cb1 matmuls; host decodes {0,1} or {-1,0,1} per block.
"""

import sys

if "/opt/trn_rl_repo" not in sys.path:
    sys.path.insert(0, "/opt/trn_rl_repo")

import numpy as np
import ml_dtypes

N_CORES = 8
N_PER_CORE = 4          # images per core
CI = 256                # in channels
CO = 256                # out channels
H = W = 56
OH = OW = 54
GP = 14                 # F(4,3) groups along W (56 out cols, last 2 cropped)
JP = 6                  # Winograd points {0, 1, -1, 2, -2, inf}
RT = 3                  # row tiles per image (18 rows each)
RROWS = 18
TFREE = RROWS * GP      # 252 outputs per j-plane per tile
JSTR = 256              # psum accumulator stride in f32 elems (1KB half-bank)
YW = 4 * TFREE          # 1008 y px per tile (56 cols incl the 2 cropped)
NT = N_PER_CORE * 2 * RT  # 24 tiles per core
N_TOT = N_CORES * N_PER_CORE
MEAN_SCALE = 1.0 / (N_TOT * OH * OW)
C_SCALE = 64.0          # residual components stored at 64x, weights at 1/64
S3 = 8.0                # host scales B^T rows 3,4 (points +-2) by 8
# head rows 0-37 (tiles 0-1), tail rows 36-55 (tile 2)
HROWS = 38
TROWS = 20
TAIL_R0 = 36
HPAD = 536              # head: 38*14=532 -> 536
TPAD = 288              # tail: 20*14=280 -> 288
NQ = 7                  # harvested sum quantities: H,V,C,E,F2,G2,ZG
QH, QV, QC, QE, QF, QG, QZ = range(NQ)
# physical j-plane order (host permutes): banks (0,1),(2,3),(4,5) hold
# logical points (m1,m3),(m4,m2),(m0,m5); matmul groups run in physical
# order so the bank a drain copy needs first is retired first
JPERM = [1, 3, 4, 2, 0, 5]   # physical slot -> logical point
JSLOT = {1: 0, 3: 1, 4: 2, 2: 3, 0: 4, 5: 5}  # logical point -> slot

# binarize engine per (cb, n, rt) chunk: v=DVE is_ge {0,1}, a=ScalarE Sign
# {-1,0,1}, g=GpSimd is_ge.  cb0 chunks run under the deferred-tile window
# where ScalarE has the most slack; cb1 chunks run post-PE where DVE's 2x
# mode makes it the fastest engine.
BIN_PAT = {0: ["aga", "vga", "aag", "aga"], 1: ["vav", "vga", "vav", "vga"]}

FP8 = ml_dtypes.float8_e4m3

# F(4,3) transforms: points {0, 1, -1, 2, -2, inf}; B^T rows 3,4 scaled by S3.
_BT = np.array([
    [1.0,    0.0,   -1.25,   0.0,    0.25,  0.0],
    [0.0,  2/3.0,   2/3.0, -1/6.0, -1/6.0,  0.0],
    [0.0, -2/3.0,   2/3.0,  1/6.0, -1/6.0,  0.0],
    [0.0, -2/3.0,  -1/3.0,  2/3.0,  1/3.0,  0.0],   # x8 applied
    [0.0,  2/3.0,  -1/3.0, -2/3.0,  1/3.0,  0.0],   # x8 applied
    [0.0,    4.0,     0.0,   -5.0,   0.0,   1.0],
], dtype=np.float64)
_G = np.array([
    [1.0,  0.0, 0.0],
    [1.0,  1.0, 1.0],
    [1.0, -1.0, 1.0],
    [1.0,  2.0, 4.0],
    [1.0, -2.0, 4.0],
    [0.0,  0.0, 1.0],
], dtype=np.float64)


def build(nc, n_cores=N_CORES):
    """Emit the SPMD program into a bacc.Bacc instance."""
    import concourse.mybir as mybir
    from concourse import tile

    f32 = mybir.dt.float32
    fp8 = mybir.dt.float8e4
    DR = mybir.MatmulPerfMode.DoubleRow
    ACT = mybir.ActivationFunctionType
    ADD = mybir.AluOpType.add
    MUL = mybir.AluOpType.mult
    IS_GE = mybir.AluOpType.is_ge

    xh_d = [
        nc.dram_tensor(f"xh{c}", [N_PER_CORE, 128, 2, JP, HPAD], fp8, kind="ExternalInput")
        for c in range(3)
    ]
    xt_d = [
        nc.dram_tensor(f"xt{c}", [N_PER_CORE, 128, 2, JP, TPAD], fp8, kind="ExternalInput")
        for c in range(3)
    ]
    w1_d = nc.dram_tensor("w1", [128, 2, JP, 3, 2, 128], fp8, kind="ExternalInput")
    ws_d = nc.dram_tensor("ws", [128, 2, JP, 3, 2, 128], fp8, kind="ExternalInput")
    y_d = nc.dram_tensor("y", [N_PER_CORE, 2, 128, RT * YW], mybir.dt.uint8, kind="ExternalOutput")

    with tile.TileContext(nc) as tc:
        with (
            tc.tile_pool(name="wpool", bufs=1) as wpool,
            tc.tile_pool(name="xpool", bufs=2) as xpool,
            tc.tile_pool(name="ypool", bufs=1) as ypool,
            tc.tile_pool(name="spool", bufs=1) as spool,
            tc.tile_pool(name="opool", bufs=1) as opool,
            tc.tile_pool(name="btpool", bufs=3) as btpool,
            tc.tile_pool(name="ctpool", bufs=2) as ctpool,
            tc.tile_pool(name="pspool", bufs=2, space="PSUM") as pspool,
            tc.tile_pool(name="drampool", bufs=2, space="DRAM") as drampool,
        ):
            w1_sb = wpool.tile([128, 2, JP, 3, 2, 128], fp8, tag="w1")
            ws_sb = wpool.tile([128, 2, JP, 3, 2, 128], fp8, tag="ws")
            y_sb = ypool.tile([128, NT * YW], f32)
            sums = spool.tile([128, NQ * NT], f32, tag="sums")
            obuf_t = opool.tile([128, N_PER_CORE * RT * YW], fp8, tag="bin")
            obuf = obuf_t[:]
            sums2 = spool.tile([128, 2, NQ], f32, tag="sums2")
            acc = spool.tile([128, 2], f32, tag="acc")
            neg_mean = spool.tile([128, 2], f32, tag="negmean")

            def emit_mean_cb(cb):
                # reduce this cb's 12 tile-columns per quantity, then combine:
                # sum(y) = F2 + G2 + H + V + 15C - 5E - ZG  (ZG = cropped cols)
                for q in range(NQ):
                    i0 = q * NT + cb * (NT // 2)
                    nc.vector.tensor_reduce(
                        sums2[:, cb, q : q + 1],
                        sums[:, i0 : i0 + NT // 2].rearrange(
                            "p (a m) -> p a m", a=1
                        ),
                        axis=mybir.AxisListType.X,
                        op=ADD,
                    )
                s2 = lambda q: sums2[:, cb, q : q + 1]
                a = acc[:, cb : cb + 1]
                nc.vector.tensor_add(a, s2(QF), s2(QG))
                nc.vector.tensor_add(a, a, s2(QH))
                nc.vector.tensor_add(a, a, s2(QV))
                nc.vector.scalar_tensor_tensor(a, s2(QC), 15.0, a, MUL, ADD)
                nc.vector.scalar_tensor_tensor(a, s2(QE), -5.0, a, MUL, ADD)
                nc.vector.scalar_tensor_tensor(a, s2(QZ), -1.0, a, MUL, ADD)
                if n_cores == 1:
                    nc.vector.tensor_scalar(
                        neg_mean[:, cb : cb + 1], a, -MEAN_SCALE, 0.0, MUL, ADD,
                    )
                else:
                    cc_in = drampool.tile([128, 1], f32, name=f"ccin{cb}")
                    cc_out = drampool.tile([128, 1], f32, name=f"ccout{cb}")
                    nc.sync.dma_start(cc_in[:], a)
                    nc.gpsimd.collective_compute(
                        "AllReduce",
                        ADD,
                        replica_groups=[list(range(n_cores))],
                        ins=[cc_in.opt()],
                        outs=[cc_out.opt()],
                    )
                    sums_g = spool.tile([128, 1], f32, tag=f"sumsg{cb}")
                    nc.sync.dma_start(sums_g[:], cc_out[:])
                    nc.vector.tensor_scalar(
                        neg_mean[:, cb : cb + 1], sums_g[:],
                        -MEAN_SCALE, 0.0, MUL, ADD,
                    )

            def emit_bin_tile(cb, bn, i, per_tile_dma=False):
                eng = BIN_PAT[cb][bn][i]
                nm = neg_mean[:, cb : cb + 1]
                bt = obuf[:, (bn * RT + i) * YW : (bn * RT + i + 1) * YW]
                ysl = y_sb[:, ((cb * N_PER_CORE + bn) * RT + i) * YW
                           : ((cb * N_PER_CORE + bn) * RT + i + 1) * YW]
                if eng == "a":
                    nc.scalar.activation(bt, ysl, ACT.Sign, bias=nm)
                elif eng == "g":
                    nc.gpsimd.tensor_scalar(bt, ysl, nm, 0.0, ADD, IS_GE)
                else:
                    nc.vector.tensor_scalar(bt, ysl, nm, 0.0, ADD, IS_GE)
                oq = (nc.sync, nc.scalar, nc.gpsimd)[bn % 3]
                if per_tile_dma:
                    oq.dma_start(
                        y_d[bn, cb][:, i * YW : (i + 1) * YW],
                        bt.bitcast(mybir.dt.uint8),
                    )
                elif i == RT - 1:
                    oq.dma_start(
                        y_d[bn, cb],
                        obuf[:, bn * RT * YW : (bn + 1) * RT * YW].bitcast(
                            mybir.dt.uint8
                        ),
                    )

            pending_bin = []

            # ---------------- phase 1: conv + drain (+sums) ------------------
            x_tiles = {}

            def load_image(n):
                xh = [
                    xpool.tile([128, 2, JP, HPAD], fp8, tag=f"xh{c}", name=f"xh{c}")
                    for c in range(3)
                ]
                xt = [
                    xpool.tile([128, 2, JP, TPAD], fp8, tag=f"xt{c}", name=f"xt{c}")
                    for c in range(3)
                ]
                if n == 0:
                    # cold start runs the TAIL tile first: it only needs
                    # w1[cb0] + the (smaller) xt planes, so the PE starts
                    # ~3us sooner while the big head planes stream in
                    nc.sync.dma_start(w1_sb[:, 0, 0], w1_d[:, 0, 0])
                    nc.scalar.dma_start(xt[0][:, :, 0:1], xt_d[0][n][:, :, 0:1])
                    nc.gpsimd.dma_start(xt[1][:, :, 0:1], xt_d[1][n][:, :, 0:1])
                    nc.sync.dma_start(ws_sb[:, 0, 0], ws_d[:, 0, 0])
                    nc.scalar.dma_start(xt[2][:, :, 0:1], xt_d[2][n][:, :, 0:1])
                    nc.gpsimd.dma_start(w1_sb[:, 0, 1:3], w1_d[:, 0, 1:3])
                    nc.sync.dma_start(xt[0][:, :, 1:3], xt_d[0][n][:, :, 1:3])
                    nc.scalar.dma_start(ws_sb[:, 0, 1:3], ws_d[:, 0, 1:3])
                    nc.gpsimd.dma_start(xt[1][:, :, 1:3], xt_d[1][n][:, :, 1:3])
                    nc.sync.dma_start(xt[2][:, :, 1:3], xt_d[2][n][:, :, 1:3])
                    nc.scalar.dma_start(w1_sb[:, 0, 3:], w1_d[:, 0, 3:])
                    nc.gpsimd.dma_start(xt[0][:, :, 3:], xt_d[0][n][:, :, 3:])
                    nc.sync.dma_start(ws_sb[:, 0, 3:], ws_d[:, 0, 3:])
                    nc.scalar.dma_start(xt[1][:, :, 3:], xt_d[1][n][:, :, 3:])
                    nc.gpsimd.dma_start(xt[2][:, :, 3:], xt_d[2][n][:, :, 3:])
                    nc.gpsimd.dma_start(xh[0][:, :, 0:3], xh_d[0][n][:, :, 0:3])
                    nc.sync.dma_start(xh[0][:, :, 3:], xh_d[0][n][:, :, 3:])
                    nc.scalar.dma_start(xh[1][:, :, 0:3], xh_d[1][n][:, :, 0:3])
                    nc.gpsimd.dma_start(xh[1][:, :, 3:], xh_d[1][n][:, :, 3:])
                    nc.sync.dma_start(xh[2][:, :, 0:3], xh_d[2][n][:, :, 0:3])
                    nc.scalar.dma_start(xh[2][:, :, 3:], xh_d[2][n][:, :, 3:])
                    nc.sync.dma_start(w1_sb[:, 1], w1_d[:, 1])
                    nc.scalar.dma_start(ws_sb[:, 1], ws_d[:, 1])
                else:
                    nc.sync.dma_start(xh[0][:], xh_d[0][n])
                    nc.scalar.dma_start(xt[0][:], xt_d[0][n])
                    nc.sync.dma_start(xh[1][:], xh_d[1][n])
                    nc.scalar.dma_start(xt[1][:], xt_d[1][n])
                    nc.sync.dma_start(xh[2][:], xh_d[2][n])
                    nc.scalar.dma_start(xt[2][:], xt_d[2][n])
                x_tiles[n] = (xh, xt)

            def emit_mm(xh, xt, ps, cb, rt, p, c, kh):
                # p is the PHYSICAL slot; host already permuted planes
                w_sb = w1_sb if c == 0 else ws_sb
                row = rt * RROWS + kh
                if rt < 2:
                    src, base = xh[c], 0
                else:
                    src, base = xt[c], TAIL_R0
                off = (row - base) * GP
                nc.tensor.matmul(
                    ps[p // 2][:, (p % 2) * JSTR : (p % 2) * JSTR + TFREE],
                    w_sb[:, cb, p, kh],
                    src[:, :, p, off : off + TFREE],
                    start=(c == 0 and kh == 0),
                    stop=(c == 2 and kh == 2),
                    perf_mode=DR,
                )

            def emit_drain(n, ps, cb, rt):
                ti = (cb * N_PER_CORE + n) * RT + rt
                q = lambda qi: sums[:, qi * NT + ti : qi * NT + ti + 1]
                mj = lambda j: ps[JSLOT[j] // 2][
                    :, (JSLOT[j] % 2) * JSTR : (JSLOT[j] % 2) * JSTR + TFREE
                ]
                ysl = lambda t: y_sb[:, (ti * 4 + t) * TFREE : (ti * 4 + t + 1) * TFREE]
                t1 = ctpool.tile([128, TFREE], f32, tag="t1", name="t1")
                t3 = ctpool.tile([128, TFREE], f32, tag="t3", name="t3")
                t4 = ctpool.tile([128, TFREE], f32, tag="t4", name="t4")
                u1 = btpool.tile([128, TFREE], f32, tag="u1", name="u1")
                v1 = btpool.tile([128, TFREE], f32, tag="v1", name="v1")
                u2 = btpool.tile([128, TFREE], f32, tag="u2", name="u2")
                v2 = btpool.tile([128, TFREE], f32, tag="v2", name="v2")
                w0 = btpool.tile([128, TFREE], f32, tag="w0", name="w0")
                w5 = btpool.tile([128, TFREE], f32, tag="w5", name="w5")
                zg = ctpool.tile([128, RROWS], f32, tag="zg", name="zg")
                # PSUM reads stay within 1 cross-engine hop of the PE so the
                # accumulator set frees ~a single op-chain after the tile's
                # last matmul: Act copies banks 0-1, DVE combines banks 1-2
                # into local w0 = m0+u1, w5 = m5+v1.  GpSimd only supports
                # the plain TensorTensor forms on HW, so the scaled combines
                # are DVE STT ops.
                nc.scalar.activation(t1[:], mj(1), ACT.Copy)
                nc.scalar.activation(t3[:], mj(3), ACT.Copy, scale=1.0 / S3, accum_out=q(QC))
                nc.scalar.activation(t4[:], mj(4), ACT.Copy, scale=1.0 / S3, accum_out=q(QE))
                nc.vector.scalar_tensor_tensor(u1[:], mj(2), 1.0, t1[:], MUL, ADD, accum_out=q(QH))
                nc.vector.scalar_tensor_tensor(v1[:], mj(2), -1.0, t1[:], MUL, ADD, accum_out=q(QV))
                nc.vector.scalar_tensor_tensor(w0[:], mj(0), 1.0, u1[:], MUL, ADD, accum_out=q(QF))
                nc.vector.scalar_tensor_tensor(w5[:], mj(5), 1.0, v1[:], MUL, ADD, accum_out=q(QG))
                nc.gpsimd.tensor_add(u2[:], t3[:], t4[:])
                nc.gpsimd.tensor_sub(v2[:], t3[:], t4[:])
                nc.gpsimd.tensor_add(ysl(0), u2[:], w0[:])
                nc.vector.scalar_tensor_tensor(ysl(1), v2[:], 2.0, v1[:], MUL, ADD)
                nc.vector.scalar_tensor_tensor(ysl(2), u2[:], 4.0, u1[:], MUL, ADD)
                nc.vector.scalar_tensor_tensor(ysl(3), v2[:], 8.0, w5[:], MUL, ADD)
                # cropped-column correction: sum of y2,y3 at group 13
                y2g = ysl(2).rearrange("p (r g) -> p r g", g=GP)[:, :, GP - 1]
                y3g = ysl(3).rearrange("p (r g) -> p r g", g=GP)[:, :, GP - 1]
                nc.vector.scalar_tensor_tensor(zg[:], y2g, 1.0, y3g, MUL, ADD, accum_out=q(QZ))

            # tile schedule: image-2's last two cb1 tiles are deferred past
            # image-3's cb0 so the post-cb0-mean window spans 5 tiles of PE
            # work for the cb0 binarize chunks to hide under
            SCHED = []
            for n in range(N_PER_CORE):
                for cb in range(2):
                    for rt in ([2, 0, 1] if (n == 0 and cb == 0) else range(RT)):
                        SCHED.append((n, cb, rt))
            SCHED.remove((2, 1, 0))
            SCHED.remove((2, 1, 1))
            SCHED.remove((2, 1, 2))
            ins = SCHED.index((3, 0, 2)) + 1
            SCHED[ins:ins] = [(2, 1, 0), (2, 1, 1), (2, 1, 2)]

            pending_bin = []
            for (n, cb, rt) in SCHED:
                if n not in x_tiles:
                    load_image(n)
                xh, xt = x_tiles[n]
                ps = [
                    pspool.tile([128, 2 * JSTR], f32, tag=f"ps{k}", name=f"ps{k}")
                    for k in range(3)
                ]
                # p-major ALWAYS: accumulation groups sharing a PSUM bank
                # must not interleave (a group's start clears the whole
                # bank's has_written bits on HW)
                for p in range(JP):
                    for c in range(3):
                        for kh in range(3):
                            emit_mm(xh, xt, ps, cb, rt, p, c, kh)
                emit_drain(n, ps, cb, rt)
                if (n, cb, rt) == (N_PER_CORE - 1, 0, RT - 1):
                    # all cb0 sums in: compute its mean now; its binarize
                    # chunks interleave with the remaining drains so every
                    # engine queue keeps the PE's PSUM sets moving
                    emit_mean_cb(0)
                    pending_bin = [
                        (0, bn, i)
                        for bn in range(N_PER_CORE)
                        for i in range(RT)
                    ]
                    for _ in range(2):
                        emit_bin_tile(*pending_bin.pop(0))
                elif pending_bin:
                    emit_bin_tile(*pending_bin.pop(0))

            # ---------------- phase 2: cb1 mean + binarize -------------------
            for args in pending_bin:
                emit_bin_tile(*args)
            emit_mean_cb(1)
            for bn in range(N_PER_CORE):
                for i in range(RT):
                    emit_bin_tile(1, bn, i, per_tile_dma=(bn == N_PER_CORE - 1))

    nc.compile()
    return nc


def prep_inputs(x, weight, bias):
    """Host-side shard + F(4,3) Winograd transform + fp8 3-comp split."""
    assert x.shape == (N_TOT, CI, H, W) and x.dtype == np.float32

    xs = np.ascontiguousarray(
        x.reshape(N_CORES, N_PER_CORE, 2, 128, H, W).transpose(0, 1, 3, 2, 4, 5)
    )  # [core, n, ci_f, ci_b, 56, 56]
    xp = np.pad(xs, ((0, 0),) * 5 + ((0, 2),))  # W 56 -> 58
    idx = np.arange(GP)[:, None] * 4 + np.arange(JP)[None, :]  # [14, 6]
    xg = xp[..., idx]  # [core, n, 128, 2, 56, 14, 6]
    D = np.einsum("jc,...gc->...gj", _BT.astype(np.float32), xg, optimize=True)
    D = np.ascontiguousarray(D.transpose(0, 1, 2, 3, 6, 4, 5)[:, :, :, :, JPERM])
    # [core, n, 128, 2, j(permuted), 56, 14]

    c1 = D.astype(FP8)
    r1 = D - c1.astype(np.float32)
    c2 = (r1 * np.float32(C_SCALE)).astype(FP8)
    r2 = r1 - c2.astype(np.float32) * np.float32(1.0 / C_SCALE)
    c3 = (r2 * np.float32(C_SCALE)).astype(FP8)

    def halves(cq):
        # [core, n, 128, 2, j, 56, 14] -> head rows 0-37, tail rows 36-55
        hd = cq[..., 0:HROWS, :].reshape(N_CORES, N_PER_CORE, 128, 2, JP, HROWS * GP)
        tl = cq[..., TAIL_R0:, :].reshape(N_CORES, N_PER_CORE, 128, 2, JP, TROWS * GP)
        hd = np.pad(hd, ((0, 0),) * 5 + ((0, HPAD - HROWS * GP),))
        tl = np.pad(tl, ((0, 0),) * 5 + ((0, TPAD - TROWS * GP),))
        return hd, tl

    hs, ts = zip(*(halves(q) for q in (c1, c2, c3)))

    wb = np.where(weight >= 0, np.float32(1.0), np.float32(-1.0))
    gt = np.einsum("jk,oihk->oihj", _G.astype(np.float32), wb, optimize=True)
    # [co, ci, kh, j] -> [ci_f, cb, j, kh, ci_b, co_f]
    g6 = gt.reshape(2, 128, 2, 128, 3, JP)
    wt = np.ascontiguousarray(g6.transpose(3, 0, 5, 4, 2, 1)[:, :, JPERM])
    w1 = wt.astype(FP8)
    ws = (wt * np.float32(1.0 / C_SCALE)).astype(FP8)
    assert np.all(w1.astype(np.float32) == wt)
    assert np.all(ws.astype(np.float32) * C_SCALE == wt)

    out = []
    for core in range(N_CORES):
        m = {"w1": w1, "ws": ws}
        for ci in range(3):
            m[f"xh{ci}"] = hs[ci][core]
            m[f"xt{ci}"] = ts[ci][core]
        out.append(m)
    return out


def gather(results):
    """[{y: [4,2,128,3024] u8}] * 8 -> (32, 256, 54, 54) fp32 +-1.

    Per tile the 1008 bytes are 4 y-slices of [18 rows x 14 groups] for
    output column 4g+t; group 13's t=2,3 are cropped.  BIN_PAT[cb][n][rt]=='a'
    chunks were binarized on ScalarE as Sign {-1,0,+1}; the rest are is_ge
    {0,1}."""
    ys = np.stack([np.asarray(r["y"]).view(FP8) for r in results]).astype(np.float32)
    ys = ys.reshape(N_CORES, N_PER_CORE, 2, 128, RT, 4, RROWS, GP)
    out = np.empty_like(ys)
    for cb in range(2):
        for n in range(N_PER_CORE):
            for i in range(RT):
                blk = ys[:, n, cb, :, i]
                out[:, n, cb, :, i] = (
                    np.where(blk < 0, -1.0, 1.0)
                    if BIN_PAT[cb][n][i] == "a"
                    else blk * 2.0 - 1.0
                )
    # [core, n, cb, co_f, rt, t, row, g] -> [core, n, cb, co_f, rt, row, g, t]
    out = out.transpose(0, 1, 2, 3, 4, 6, 7, 5).reshape(
        N_CORES, N_PER_CORE, CO, OH, 4 * GP
    )[..., :OW]
    return np.ascontiguousarray(out.reshape(N_TOT, CO, OH, OW))


_STATE = {}


def _get_nc():
    if "nc" not in _STATE:
        import concourse.bacc as bacc

        nc = bacc.Bacc(
            "TRN2", target_bir_lowering=False, debug=False, num_devices=N_CORES
        )
        _STATE["nc"] = build(nc)
    return _STATE["nc"]


def kernel(x, weight, bias, _trace=False):
    from concourse.bass_utils import run_bass_kernel_spmd

    nc = _get_nc()
    in_maps = prep_inputs(
        np.asarray(x, np.float32),
        np.asarray(weight, np.float32),
        np.asarray(bias, np.float32),
    )
    res = run_bass_kernel_spmd(
        nc, in_maps, core_ids=list(range(N_CORES)), trace=_trace
    )
    _STATE["last_result"] = res
    return gather(res.results)
